# revision 1
# baseline (speedup 1.0000x reference)
"""Trainium2 Bass kernel for BEiT attention block (nn_Beit_9560597201107).

Data-parallel over batch: 64 batches -> 8 NeuronCores x 8 batches each.
Fully transposed dataflow (channels on partitions) so the softmax'd
attention matrix is never transposed on-chip:

  xT = x.T (PE transpose)                          [768, 197]
  qkT[c, n] = sum_k WT[k, c] xT[k, n] + bias       [1536, 197]  (q pre-scaled)
  v[m, d]   = sum_k xT[k, m] WT_v[k, d] + bias     [197, 768]   (natural)
  scT[m, n] = sum_d kT[d, m] qT[d, n]              per head
  eT = exp(scT) * exp_rel_T                        (rel bias via exp-mult)
  sums[h, n] = sum_m eT[m, n]   (ones-column matmul)
  po[d, n]  = sum_m v[m, d] eT[m, n]               (unnormalized outT)
  cT = po * broadcast(1/sums)   (PE ones-outer-product broadcast)
  y[n, o] = sum_c cT[c, n] projWT[c, o] + bias

All matmuls run in float32r (full-rate fp32, ~1e-4 relative rounding);
free dims padded to 256 to stay at 1 cycle/row.
"""

import os
import numpy as np

import concourse.bass as bass
import concourse.bacc as bacc
import concourse.mybir as mybir
import concourse.tile as tile
from concourse.bass_utils import run_bass_kernel_spmd
from concourse.bass_interp import get_hw_module
B, N, DIM, HEADS, NBS = 64, 197, 768, 12, 10
HEAD_DIM = DIM // HEADS
SCALE = HEAD_DIM ** -0.5
NCORES = 8
BPC = B // NCORES          # batches per core
KT = DIM // 128            # 6 contraction tiles
NPAD = 256                 # padded token free-dim (fp32r needs >=256 for full rate)
TOK_TILES = [(0, 128), (128, 69)]  # (offset, size) over the 197 tokens
# Scores head-pairs grouped by parity: both heads of a pair live at the same
# 64-partition half of qkT, so their back-to-back matmuls into one PSUM bank
# use the same PE row group (mixed row groups on one bank crash fp32r).
PAIRS = [(0, 2), (4, 6), (8, 10), (1, 3), (5, 7), (9, 11)]
PAIR_PERM = [h for p in PAIRS for h in p]

F32 = mybir.dt.float32
F32R = mybir.dt.float32r

_CACHE = {}


def _build_module():
    nc = bacc.Bacc("TRN2", target_bir_lowering=False, debug=False)

    # host-transposed, zero-padded x: xt8[b, k, p, n] = x[b, n, 128k+p]
    xt8_d = nc.dram_tensor("xt8", [BPC, KT, 128, NPAD], F32, kind="ExternalInput")
    wt_d = nc.dram_tensor("wt", [KT, 128, 3 * DIM], F32, kind="ExternalInput")
    pwt_d = nc.dram_tensor("pwt", [KT, 128, DIM], F32, kind="ExternalInput")
    qbc_d = nc.dram_tensor("qbc", [128, BPC, KT], F32, kind="ExternalInput")
    vpb_d = nc.dram_tensor("vpb8", [BPC, DIM], F32, kind="ExternalInput")
    relt_d = nc.dram_tensor("relt", [6, 2, 128, 2 * NPAD], F32, kind="ExternalInput")
    ones_d = nc.dram_tensor("ones1", [1, NPAD], F32, kind="ExternalInput")
    oh3_d = nc.dram_tensor("oh3", [128, 3, 65], F32, kind="ExternalInput")
    allones_d = nc.dram_tensor("allones", [128, 64], F32, kind="ExternalInput")
    y8_d = nc.dram_tensor("y8", [BPC, N, DIM], F32, kind="ExternalOutput")

    with tile.TileContext(nc) as tc:
        with (
            tc.tile_pool(name="const", bufs=1) as constp,
            tc.tile_pool(name="sb_xT", bufs=1) as sb_xT,
            tc.tile_pool(name="sb_qkT", bufs=1) as sb_qkT,
            tc.tile_pool(name="sb_v", bufs=2) as sb_v,
            tc.tile_pool(name="sb_exp", bufs=2) as sb_exp,
            tc.tile_pool(name="sb_po", bufs=8) as sb_po,
            tc.tile_pool(name="sb_ctmp", bufs=2) as sb_ctmp,
            tc.tile_pool(name="sb_pbs", bufs=2) as sb_pbs,
            tc.tile_pool(name="sb_rec", bufs=2) as sb_rec,
            tc.tile_pool(name="sb_cT", bufs=2) as sb_cT,
            tc.tile_pool(name="sb_out", bufs=2) as sb_out,
            tc.tile_pool(name="sb_vpb", bufs=2) as sb_vpb,
            tc.tile_pool(name="ps", bufs=6, space="PSUM") as ps,
            tc.tile_pool(name="ps_sums", bufs=2, space="PSUM") as ps_sums,
        ):
            # ---- persistent data (loaded once) ----
            wt_sb = constp.tile([128, KT, 3 * DIM], F32R)
            nc.gpsimd.dma_start(out=wt_sb[:], in_=wt_d.ap().transpose([1, 0, 2]))
            pwt_sb = constp.tile([128, KT, DIM], F32R)
            nc.gpsimd.dma_start(out=pwt_sb[:], in_=pwt_d.ap().transpose([1, 0, 2]))
            relt_sb = constp.tile([128, 6, 2, 2 * NPAD], F32R)
            nc.gpsimd.dma_start(out=relt_sb[:], in_=relt_d.ap().transpose([2, 0, 1, 3]))
            qbc_sb = constp.tile([128, BPC, KT], F32)
            nc.sync.dma_start(out=qbc_sb[:], in_=qbc_d.ap())

            ones_sb = constp.tile([1, NPAD], F32R)
            nc.gpsimd.dma_start(out=ones_sb[:], in_=ones_d.ap())
            oh3_sb = constp.tile([128, 3, 65], F32R)
            nc.gpsimd.dma_start(out=oh3_sb[:], in_=oh3_d.ap())
            allones_sb = constp.tile([128, 64], F32R)
            nc.gpsimd.dma_start(out=allones_sb[:], in_=allones_d.ap())

            def kT(qkT_sb, h, hb, off, mt):
                base = (h % 2) * 64
                return qkT_sb[base:base + 64, 6 + h // 2,
                              hb * NPAD + off:hb * NPAD + off + mt]

            def qT(qkT_sb, h, hb):
                base = (h % 2) * 64
                return qkT_sb[base:base + 64, h // 2, hb * NPAD:(hb + 1) * NPAD]

            prev_proj = [None]
            proj_state = {}

            def emit_proj_chunk(step, drain=False):
                if prev_proj[0] is None:
                    return
                pb_, cT_, vpb_ = prev_proj[0]
                if step == 0 and not drain:
                    proj_state.clear()
                chunks = [(0, 0), (0, 1), (0, 2), (1, 0), (1, 1), (1, 2)]
                todo = chunks if drain else [chunks[step]]
                for (t, jc) in todo:
                    off, mt = TOK_TILES[t]
                    if jc == 0:
                        proj_state[t] = (
                            ps.tile([128, 512], F32, tag="ps", name=f"pr_{pb_}_{t}"),
                            ps.tile([128, NPAD], F32, tag="ps", name=f"pr2_{pb_}_{t}"),
                        )
                    pr, pr2 = proj_state[t]
                    for j in (2 * jc, 2 * jc + 1):
                        nc.tensor.matmul(
                            pr[0:mt, :], cT_[:, j, off:off + mt], pwt_sb[:, j, 0:512],
                            start=(j == 0), stop=False,
                        )
                        nc.tensor.matmul(
                            pr2[0:mt, :], cT_[:, j, off:off + mt], pwt_sb[:, j, 512:768],
                            start=(j == 0), stop=False,
                        )
                    if jc == 2:
                        nc.tensor.matmul(
                            pr[0:mt, :], ones_sb[0:1, 0:mt], vpb_[0:1, 0:512],
                            start=False, stop=True,
                        )
                        nc.tensor.matmul(
                            pr2[0:mt, :], ones_sb[0:1, 0:mt], vpb_[0:1, 512:768],
                            start=False, stop=True,
                        )
                        out_sb = sb_out.tile([128, DIM], F32, tag="out",
                                             name=f"out_{pb_}_{t}")
                        nc.scalar.copy(out_sb[0:mt, 0:512], pr[0:mt, :])
                        nc.vector.tensor_copy(out_sb[0:mt, 512:768], pr2[0:mt, :])
                        nc.sync.dma_start(out=y8_d.ap()[pb_, off:off + mt, :],
                                          in_=out_sb[0:mt, :])
                if drain:
                    prev_proj[0] = None

            for g in range(BPC // 2):
                # ---- load host-transposed x for the batch pair ----
                xT_sb = sb_xT.tile([128, KT, 2 * NPAD], F32R, tag="xT", name=f"xT_{g}")
                for hb in range(2):
                    nc.gpsimd.dma_start(
                        out=xT_sb[:, :, hb * NPAD:(hb + 1) * NPAD],
                        in_=xt8_d.ap()[2 * g + hb].transpose([1, 0, 2]),
                    )

                # ---- qkT for both batches (one weight load per block) ----
                qkT_sb = sb_qkT.tile([128, 12, 2 * NPAD], F32R, tag="qkT", name=f"qkT_{g}")
                for ct in range(12):
                    qp = ps.tile([128, 512], F32, tag="ps", name=f"qp_{g}_{ct}")
                    for k in range(KT):
                        nc.tensor.matmul(
                            qp[:],
                            wt_sb[:, k, ct * 128:(ct + 1) * 128],
                            xT_sb[:, k, :],
                            start=(k == 0),
                            stop=(k == KT - 1),
                        )
                    if ct < 6:
                        for hb in range(2):
                            qbias = qbc_sb[:, 2 * g + hb, ct:ct + 1]
                            dst = qkT_sb[:, ct, hb * NPAD:(hb + 1) * NPAD]
                            srcp = qp[:, hb * NPAD:(hb + 1) * NPAD]
                            if ct % 2 == 0:
                                nc.vector.tensor_scalar_add(dst, srcp, qbias)
                            else:
                                nc.scalar.activation(
                                    dst, srcp,
                                    mybir.ActivationFunctionType.Identity, bias=qbias,
                                )
                    else:
                        if ct % 2 == 0:
                            nc.vector.tensor_copy(qkT_sb[:, ct, :], qp[:])
                        else:
                            nc.scalar.copy(qkT_sb[:, ct, :], qp[:])

                for hb in range(2):
                    b = 2 * g + hb

                    vpb_t = sb_vpb.tile([1, DIM], F32R, tag="vpb", name=f"vpb_{b}")
                    nc.gpsimd.dma_start(out=vpb_t[:], in_=vpb_d.ap()[b].unsqueeze(0))

                    # ---- v (natural layout) ----
                    v_sb = sb_v.tile([128, 2, HEADS, HEAD_DIM], F32R, tag="v",
                                     name=f"v_{b}")
                    for t, (off, mt) in enumerate(TOK_TILES):
                        vp = ps.tile([128, 512], F32, tag="ps", name=f"vp_{b}_{t}")
                        vp2 = ps.tile([128, NPAD], F32, tag="ps", name=f"vp2_{b}_{t}")
                        for k in range(KT):
                            xsl = xT_sb[:, k, hb * NPAD + off:hb * NPAD + off + mt]
                            nc.tensor.matmul(
                                vp[0:mt, :], xsl, wt_sb[:, k, 1536:2048],
                                start=(k == 0), stop=(k == KT - 1),
                            )
                            nc.tensor.matmul(
                                vp2[0:mt, :], xsl, wt_sb[:, k, 2048:2304],
                                start=(k == 0), stop=(k == KT - 1),
                            )
                        # v_sb head axis is in PAIR_PERM order: even head h ->
                        # slot h//2, odd head h -> slot 6 + h//2
                        nc.vector.tensor_copy(
                            v_sb[0:mt, t, :, :].rearrange(
                                "p (par a) d -> p a par d", par=2)[:, 0:4, :, :],
                            vp[0:mt, :].rearrange("p (a par d) -> p a par d",
                                                  par=2, d=HEAD_DIM),
                        )
                        nc.scalar.copy(
                            v_sb[0:mt, t, :, :].rearrange(
                                "p (par a) d -> p a par d", par=2)[:, 4:6, :, :],
                            vp2[0:mt, :].rearrange("p (a par d) -> p a par d",
                                                   par=2, d=HEAD_DIM),
                        )

                    # ---- attention, with prev-batch proj interleaved ----
                    sums_pA = ps_sums.tile([65, 512], F32, tag="sums", name=f"sumsA_{b}")
                    sums_pB = ps_sums.tile([65, 512], F32, tag="sums", name=f"sumsB_{b}")
                    po_sb_by_pair = {}
                    for step, sp in enumerate([0, 3, 1, 4, 2, 5]):
                        h0, h1 = PAIRS[sp]
                        expT = sb_exp.tile([128, 2, 2 * NPAD], F32R, tag="expT",
                                           name=f"expT_{b}_{sp}")
                        sums_px = sums_pA if sp < 3 else sums_pB
                        j3 = sp % 3
                        scs = []
                        for t, (off, mt) in enumerate(TOK_TILES):
                            sc = ps.tile([128, 512], F32, tag="ps", name=f"sc_{b}_{sp}_{t}")
                            nc.tensor.matmul(
                                sc[0:mt, 0:NPAD], kT(qkT_sb, h0, hb, off, mt),
                                qT(qkT_sb, h0, hb), start=True, stop=True,
                            )
                            nc.tensor.matmul(
                                sc[0:mt, NPAD:512], kT(qkT_sb, h1, hb, off, mt),
                                qT(qkT_sb, h1, hb), start=True, stop=True,
                            )
                            scs.append(sc)
                        for t, (off, mt) in enumerate(TOK_TILES):
                            nc.scalar.activation(
                                expT[0:mt, t, :], scs[t][0:mt, :],
                                mybir.ActivationFunctionType.Exp,
                            )
                            nc.vector.tensor_mul(
                                expT[0:mt, t, :], expT[0:mt, t, :],
                                relt_sb[0:mt, sp, t, :],
                            )
                        # long warm proj matmuls of the previous batch fill the
                        # exp/mult wait and keep the PE clock-gate open
                        emit_proj_chunk(step)
                        po = ps.tile([128, 512], F32, tag="ps", name=f"po_{b}_{sp}")
                        for t, (off, mt) in enumerate(TOK_TILES):
                            nc.tensor.matmul(
                                po[:, :], v_sb[0:mt, t, 2 * sp:2 * sp + 2, :],
                                expT[0:mt, t, :], start=(t == 0), stop=(t == 1),
                            )
                            nc.tensor.matmul(
                                sums_px[0:65, :], oh3_sb[0:mt, j3, :], expT[0:mt, t, :],
                                start=(sp in (0, 3) and t == 0),
                                stop=(sp in (2, 5) and t == 1),
                                skip_group_check=True,
                            )
                        po_sb = sb_po.tile([128, NPAD], F32, tag="po",
                                           name=f"po_sb_{b}_{sp}")
                        nc.scalar.copy(po_sb[0:64, :], po[0:64, 0:NPAD])
                        nc.vector.tensor_copy(po_sb[64:128, :], po[64:128, NPAD:512])
                        po_sb_by_pair[sp] = po_sb

                    rec_fA = sb_rec.tile([65, 512], F32, tag="recf", name=f"recfA_{b}")
                    rec_fB = sb_rec.tile([65, 512], F32, tag="recf", name=f"recfB_{b}")
                    nc.vector.reciprocal_approx_fast(out=rec_fA[0:65, :],
                                                     in_=sums_pA[0:65, :])
                    nc.vector.reciprocal_approx_fast(out=rec_fB[0:65, :],
                                                     in_=sums_pB[0:65, :])
                    rec_sbA = sb_rec.tile([65, 512], F32R, tag="rec", name=f"recA_{b}")
                    rec_sbB = sb_rec.tile([65, 512], F32R, tag="rec", name=f"recB_{b}")
                    nc.scalar.copy(rec_sbA[0:65, :], rec_fA[0:65, :])
                    nc.scalar.copy(rec_sbB[0:65, :], rec_fB[0:65, :])

                    cT_sb = sb_cT.tile([128, KT, NPAD], F32R, tag="cT", name=f"cT_{b}")
                    for sp in range(6):
                        rec_x = rec_sbA if sp < 3 else rec_sbB
                        r0 = 32 * (sp % 3)
                        pb2 = ps.tile([64, 512], F32, tag="ps", name=f"pb_{b}_{sp}")
                        nc.tensor.matmul(
                            pb2[0:64, :], allones_sb[r0:r0 + 1, 0:64],
                            rec_x[r0:r0 + 1, 0:512],
                            start=True, stop=True,
                        )
                        pblo = sb_ctmp.tile([64, NPAD], F32, tag="ctmp",
                                            name=f"pblo_{b}_{sp}")
                        nc.scalar.copy(pblo[:], pb2[0:64, NPAD:512])
                        pbhi = sb_pbs.tile([128, NPAD], F32, tag="pbs",
                                           name=f"pbhi_{b}_{sp}")
                        nc.sync.dma_start(out=pbhi[64:128, :], in_=pblo[:])
                        po_sb = po_sb_by_pair[sp]
                        nc.vector.tensor_mul(cT_sb[0:64, sp, :], po_sb[0:64, :],
                                             pb2[0:64, 0:NPAD])
                        nc.vector.tensor_mul(cT_sb[64:128, sp, :], po_sb[64:128, :],
                                             pbhi[64:128, :])
                    prev_proj[0] = (b, cT_sb, vpb_t)

            # drain the last batch's projection
            emit_proj_chunk(0, drain=True)

    nc.compile()
    nc.m = get_hw_module(nc.m)
    return nc


def _host_prep(x, qkv_weight, q_bias, v_bias, rel_table, proj_weight, proj_bias,
               b_idx, rel_index):
    x = np.asarray(x, dtype=np.float32)
    # xt8[b, k, p, n] = x[b, n, 128k+p], zero-padded to NPAD tokens
    xt = np.zeros((B, KT, 128, NPAD), dtype=np.float32)
    xt[:, :, :, 0:N] = x.transpose(0, 2, 1).reshape(B, KT, 128, N)
    W = np.asarray(qkv_weight, dtype=np.float32).copy()
    W[:DIM] *= np.float32(SCALE)
    wt = np.ascontiguousarray(W.T.reshape(KT, 128, 3 * DIM))
    pwtT = np.asarray(proj_weight, dtype=np.float32).T  # [c', o]
    pwtT = pwtT.reshape(HEADS, HEAD_DIM, DIM)[PAIR_PERM].reshape(DIM, DIM)
    pwt = np.ascontiguousarray(pwtT.reshape(KT, 128, DIM))

    bi = np.asarray(b_idx).astype(np.int64)
    qb_all = (np.asarray(q_bias, dtype=np.float32)[bi] * np.float32(SCALE))
    vb_all = np.asarray(v_bias, dtype=np.float32)[bi]
    # softmax rows sum to 1, so attn @ (1 x vb) == 1 x vb; push the v bias
    # through the projection into the proj bias
    pb_all = (np.asarray(proj_bias, dtype=np.float32)[bi]
              + vb_all @ np.asarray(proj_weight, dtype=np.float32).T)

    ridx = np.asarray(rel_index).astype(np.int64)
    rel = np.asarray(rel_table, dtype=np.float32)[ridx.reshape(-1)]
    rel = rel.reshape(N, N, HEADS)  # [n, m, h]
    relth = np.zeros((HEADS, 2, 128, NPAD), dtype=np.float32)
    for t, (off, mt) in enumerate(TOK_TILES):
        # relth[h, t, p, n] = exp(rel[n, off+p, h])
        relth[:, t, 0:mt, 0:N] = np.exp(rel[:, off:off + mt, :].transpose(2, 1, 0))
    # pair-merged: relt[sp, t, p, i*NPAD+n] = relth[PAIRS[sp][i], t, p, n]
    relt = np.ascontiguousarray(
        relth[PAIR_PERM].reshape(6, 2, 2, 128, NPAD)
        .transpose(0, 2, 3, 1, 4).reshape(6, 2, 128, 2 * NPAD))

    ones1 = np.zeros((1, NPAD), dtype=np.float32)
    ones1[0, 0:N] = 1.0
    oh3 = np.zeros((128, 3, 65), dtype=np.float32)
    for j in range(3):
        oh3[:, j, 32 * j] = 1.0
    allones = np.ones((128, 64), dtype=np.float32)

    in_maps = []
    for c in range(NCORES):
        sl = slice(c * BPC, (c + 1) * BPC)
        qbc = np.ascontiguousarray(
            qb_all[sl].reshape(BPC, KT, 128).transpose(2, 0, 1))
        vpb = np.ascontiguousarray(pb_all[sl])
        in_maps.append({
            "xt8": np.ascontiguousarray(xt[sl]),
            "wt": wt,
            "pwt": pwt,
            "qbc": qbc,
            "vpb8": vpb,
            "relt": relt,
            "ones1": ones1,
            "oh3": oh3,
            "allones": allones,
        })
    return in_maps


def _install_ntff_hook():
    """Provide antenv.axon_hooks (absent from this image) so bass_utils can
    capture NTFF profiles through libaxon_pjrt.so, and keep artifacts local."""
    if _CACHE.get("hook_installed"):
        return
    import sys
    import types
    import ctypes
    import contextlib

    so_path = "/opt/axon/libaxon_pjrt.so"
    lib = ctypes.CDLL(so_path)
    lib.axon_start_nrt_profile.argtypes = [
        ctypes.POINTER(ctypes.c_int64),
        ctypes.c_size_t,
    ]
    lib.axon_start_nrt_profile.restype = ctypes.c_int64
    lib.axon_stop_nrt_profile.argtypes = [ctypes.c_char_p]
    lib.axon_stop_nrt_profile.restype = ctypes.c_int64

    @contextlib.contextmanager
    def _hook(output_dir, device_ids):
        import jax

        jax.devices()
        if device_ids:
            ids = (ctypes.c_int64 * len(device_ids))(*device_ids)
            rc = lib.axon_start_nrt_profile(ids, len(device_ids))
        else:
            rc = lib.axon_start_nrt_profile(None, 0)
        if rc != 0:
            raise RuntimeError(f"axon_start_nrt_profile rc={rc}")
        try:
            yield
        finally:
            n = lib.axon_stop_nrt_profile(str(output_dir).encode())
            print(f"ntff profile: {n} file(s) written to {output_dir}")

    mod = types.ModuleType("antenv.axon_hooks")
    mod.get_axon_ntff_profile_hook = lambda: _hook
    mod.set_axon_ntff_profile_hook = lambda h: None
    sys.modules["antenv.axon_hooks"] = mod

    import concourse.bass_utils as bu

    bu.upload_artifacts = lambda tmpdir: str(tmpdir)
    _CACHE["hook_installed"] = True


def kernel(**inputs):
    if "nc" not in _CACHE:
        _CACHE["nc"] = _build_module()
    nc = _CACHE["nc"]

    in_maps = _host_prep(**inputs)
    trace = os.environ.get("KERNEL_TRACE", "0") == "1"
    tmpdir = None
    if trace:
        _install_ntff_hook()
        tmpdir = os.environ.get("KERNEL_TRACE_DIR") or None
    res = run_bass_kernel_spmd(nc, in_maps, core_ids=list(range(NCORES)), trace=trace,
                               tmpdir=tmpdir)
    if trace:
        _CACHE["last_exec_time_ns"] = res.exec_time_ns
        _CACHE["last_results"] = res

    y = np.concatenate([res.results[c]["y8"] for c in range(NCORES)], axis=0)
    return y



# revision 8
# speedup vs baseline: 1.0464x; 1.0464x over previous
"""Trainium2 Bass kernel for BEiT attention block (nn_Beit_9560597201107).

Data-parallel over batch: 64 batches -> 8 NeuronCores x 8 batches each.
Fully transposed dataflow (channels on partitions) so the softmax'd
attention matrix is never transposed on-chip:

  xT = x.T (PE transpose)                          [768, 197]
  qkT[c, n] = sum_k WT[k, c] xT[k, n] + bias       [1536, 197]  (q pre-scaled)
  v[m, d]   = sum_k xT[k, m] WT_v[k, d] + bias     [197, 768]   (natural)
  scT[m, n] = sum_d kT[d, m] qT[d, n]              per head
  eT = exp(scT) * exp_rel_T                        (rel bias via exp-mult)
  sums[h, n] = sum_m eT[m, n]   (ones-column matmul)
  po[d, n]  = sum_m v[m, d] eT[m, n]               (unnormalized outT)
  cT = po * broadcast(1/sums)   (PE ones-outer-product broadcast)
  y[n, o] = sum_c cT[c, n] projWT[c, o] + bias

All matmuls run in float32r (full-rate fp32, ~1e-4 relative rounding);
free dims padded to 256 to stay at 1 cycle/row.
"""

import os
import numpy as np

import concourse.bass as bass
import concourse.bacc as bacc
import concourse.mybir as mybir
import concourse.tile as tile
from concourse.bass_utils import run_bass_kernel_spmd
from concourse.bass_interp import get_hw_module
B, N, DIM, HEADS, NBS = 64, 197, 768, 12, 10
HEAD_DIM = DIM // HEADS
SCALE = HEAD_DIM ** -0.5
NCORES = 8
BPC = B // NCORES          # batches per core
KT = DIM // 128            # 6 contraction tiles
NPAD = 256                 # padded token free-dim (fp32r needs >=256 for full rate)
TOK_TILES = [(0, 128), (128, 69)]  # (offset, size) over the 197 tokens
# Scores head-pairs grouped by parity: both heads of a pair live at the same
# 64-partition half of qkT, so their back-to-back matmuls into one PSUM bank
# use the same PE row group (mixed row groups on one bank crash fp32r).
PAIRS = [(0, 2), (4, 6), (8, 10), (1, 3), (5, 7), (9, 11)]
PAIR_PERM = [h for p in PAIRS for h in p]

F32 = mybir.dt.float32
F32R = mybir.dt.float32r

_CACHE = {}


def _build_module():
    nc = bacc.Bacc("TRN2", target_bir_lowering=False, debug=False)

    # host-transposed, zero-padded x: xt8[b, k, p, n] = x[b, n, 128k+p]
    xt8_d = nc.dram_tensor("xt8", [BPC, KT, 128, NPAD], F32, kind="ExternalInput")
    wt_d = nc.dram_tensor("wt", [KT, 128, 3 * DIM], F32, kind="ExternalInput")
    pwt_d = nc.dram_tensor("pwt", [KT, 128, DIM], F32, kind="ExternalInput")
    qbc_d = nc.dram_tensor("qbc", [128, BPC, KT], F32, kind="ExternalInput")
    relt_d = nc.dram_tensor("relt", [6, 2, 128, 2 * NPAD], F32, kind="ExternalInput")
    oh3_d = nc.dram_tensor("oh3", [128, 3, 65], F32, kind="ExternalInput")
    allones_d = nc.dram_tensor("allones", [128, 64], F32, kind="ExternalInput")
    y8_d = nc.dram_tensor("y8", [BPC, N, DIM], F32, kind="ExternalOutput")

    with tile.TileContext(nc) as tc:
        with (
            tc.tile_pool(name="const", bufs=1) as constp,
            tc.tile_pool(name="sb_xT", bufs=1) as sb_xT,
            tc.tile_pool(name="sb_qkT", bufs=1) as sb_qkT,
            tc.tile_pool(name="sb_v", bufs=2) as sb_v,
            tc.tile_pool(name="sb_exp", bufs=2) as sb_exp,
            tc.tile_pool(name="sb_po", bufs=8) as sb_po,
            tc.tile_pool(name="sb_ctmp", bufs=2) as sb_ctmp,
            tc.tile_pool(name="sb_pbs", bufs=2) as sb_pbs,
            tc.tile_pool(name="sb_rec", bufs=2) as sb_rec,
            tc.tile_pool(name="sb_cT", bufs=2) as sb_cT,
            tc.tile_pool(name="sb_out", bufs=2) as sb_out,
            tc.tile_pool(name="ps", bufs=6, space="PSUM") as ps,
            tc.tile_pool(name="ps_sums", bufs=2, space="PSUM") as ps_sums,
        ):
            # ---- persistent data, streamed in consumption order so the
            # first qkT matmuls start ~5us in instead of waiting for the
            # whole 12MB of constants ----
            wt_sb = constp.tile([128, KT, 3 * DIM], F32R)
            # q+k column chunks first (ct 0..11 of the qkT loop)
            for ct in range(12):
                nc.gpsimd.dma_start(
                    out=wt_sb[:, :, ct * 128:(ct + 1) * 128],
                    in_=wt_d.ap().transpose([1, 0, 2])[:, :, ct * 128:(ct + 1) * 128],
                )
            qbc_sb = constp.tile([128, BPC, KT], F32)
            nc.sync.dma_start(out=qbc_sb[:], in_=qbc_d.ap())
            oh3_sb = constp.tile([128, 3, 65], F32R)
            nc.gpsimd.dma_start(out=oh3_sb[:], in_=oh3_d.ap())
            allones_sb = constp.tile([128, 64], F32R)
            nc.gpsimd.dma_start(out=allones_sb[:], in_=allones_d.ap())

            # first batch-pair of x right behind the q/k weights
            xT_tiles = {}

            def load_xT(g):
                t_ = sb_xT.tile([128, KT, 2 * NPAD], F32R, tag="xT", name=f"xT_{g}")
                for hb in range(2):
                    nc.gpsimd.dma_start(
                        out=t_[:, :, hb * NPAD:(hb + 1) * NPAD],
                        in_=xt8_d.ap()[2 * g + hb].transpose([1, 0, 2]),
                    )
                xT_tiles[g] = t_

            load_xT(0)

            # v weight chunks (needed right after qkT of pair 0)
            for ct in range(12, 18):
                nc.gpsimd.dma_start(
                    out=wt_sb[:, :, ct * 128:(ct + 1) * 128],
                    in_=wt_d.ap().transpose([1, 0, 2])[:, :, ct * 128:(ct + 1) * 128],
                )
            # rel-pos table (needed at the first exp, ~10us in)
            relt_sb = constp.tile([128, 6, 2, 2 * NPAD], F32R)
            nc.gpsimd.dma_start(out=relt_sb[:], in_=relt_d.ap().transpose([2, 0, 1, 3]))
            load_xT(1)
            # proj weights (first used ~25us in, interleaved into pair 1)
            pwt_sb = constp.tile([128, KT, DIM], F32R)
            nc.gpsimd.dma_start(out=pwt_sb[:], in_=pwt_d.ap().transpose([1, 0, 2]))

            def kT(qkT_sb, h, hb, off, mt):
                base = (h % 2) * 64
                return qkT_sb[base:base + 64, 6 + h // 2,
                              hb * NPAD + off:hb * NPAD + off + mt]

            def qT(qkT_sb, h, hb):
                base = (h % 2) * 64
                return qkT_sb[base:base + 64, h // 2, hb * NPAD:(hb + 1) * NPAD]

            prev_proj = [None]
            proj_state = {}

            def emit_proj_chunk(step, drain=False):
                if prev_proj[0] is None:
                    return
                pb_, cT_ = prev_proj[0]
                if step == 0 and not drain:
                    proj_state.clear()
                chunks = [(0, 0), (0, 1), (0, 2), (1, 0), (1, 1), (1, 2)]
                todo = chunks if drain else [chunks[step]]
                for (t, jc) in todo:
                    off, mt = TOK_TILES[t]
                    if jc == 0:
                        proj_state[t] = (
                            ps.tile([128, 512], F32, tag="ps", name=f"pr_{pb_}_{t}"),
                            ps.tile([128, NPAD], F32, tag="ps", name=f"pr2_{pb_}_{t}"),
                        )
                    pr, pr2 = proj_state[t]
                    for j in (2 * jc, 2 * jc + 1):
                        nc.tensor.matmul(
                            pr[0:mt, :], cT_[:, j, off:off + mt], pwt_sb[:, j, 0:512],
                            start=(j == 0), stop=(j == 5),
                        )
                        nc.tensor.matmul(
                            pr2[0:mt, :], cT_[:, j, off:off + mt], pwt_sb[:, j, 512:768],
                            start=(j == 0), stop=(j == 5),
                        )
                    if jc == 2:
                        out_sb = sb_out.tile([128, DIM], F32, tag="out",
                                             name=f"out_{pb_}_{t}")
                        nc.scalar.copy(out_sb[0:mt, 0:512], pr[0:mt, :])
                        nc.vector.tensor_copy(out_sb[0:mt, 512:768], pr2[0:mt, :])
                        nc.sync.dma_start(out=y8_d.ap()[pb_, off:off + mt, :],
                                          in_=out_sb[0:mt, :])
                if drain:
                    prev_proj[0] = None

            for g in range(BPC // 2):
                # ---- host-transposed x for the batch pair (prefetched) ----
                xT_sb = xT_tiles.pop(g)

                # ---- qkT for both batches (one weight load per block) ----
                qkT_sb = sb_qkT.tile([128, 12, 2 * NPAD], F32R, tag="qkT", name=f"qkT_{g}")
                for ct in range(12):
                    qp = ps.tile([128, 512], F32, tag="ps", name=f"qp_{g}_{ct}")
                    for k in range(KT):
                        nc.tensor.matmul(
                            qp[:],
                            wt_sb[:, k, ct * 128:(ct + 1) * 128],
                            xT_sb[:, k, :],
                            start=(k == 0),
                            stop=(k == KT - 1),
                        )
                    if ct < 6:
                        for hb in range(2):
                            qbias = qbc_sb[:, 2 * g + hb, ct:ct + 1]
                            dst = qkT_sb[:, ct, hb * NPAD:(hb + 1) * NPAD]
                            srcp = qp[:, hb * NPAD:(hb + 1) * NPAD]
                            if ct % 2 == 0:
                                nc.vector.tensor_scalar_add(dst, srcp, qbias)
                            else:
                                nc.scalar.activation(
                                    dst, srcp,
                                    mybir.ActivationFunctionType.Identity, bias=qbias,
                                )
                    else:
                        if ct % 2 == 0:
                            nc.vector.tensor_copy(qkT_sb[:, ct, :], qp[:])
                        else:
                            nc.scalar.copy(qkT_sb[:, ct, :], qp[:])

                # prefetch next-next pair's x while this pair computes
                if g + 2 < BPC // 2:
                    load_xT(g + 2)

                v_tiles = {}

                def v_phase(hb):
                    b = 2 * g + hb

                    # ---- v (natural layout) ----
                    v_sb = sb_v.tile([128, 2, HEADS, HEAD_DIM], F32R, tag="v",
                                     name=f"v_{b}")
                    v_tiles[hb] = v_sb
                    for t, (off, mt) in enumerate(TOK_TILES):
                        vp = ps.tile([128, 512], F32, tag="ps", name=f"vp_{b}_{t}")
                        vp2 = ps.tile([128, NPAD], F32, tag="ps", name=f"vp2_{b}_{t}")
                        for k in range(KT):
                            xsl = xT_sb[:, k, hb * NPAD + off:hb * NPAD + off + mt]
                            nc.tensor.matmul(
                                vp[0:mt, :], xsl, wt_sb[:, k, 1536:2048],
                                start=(k == 0), stop=(k == KT - 1),
                            )
                            nc.tensor.matmul(
                                vp2[0:mt, :], xsl, wt_sb[:, k, 2048:2304],
                                start=(k == 0), stop=(k == KT - 1),
                            )
                        # v_sb head axis is in PAIR_PERM order: even head h ->
                        # slot h//2, odd head h -> slot 6 + h//2
                        nc.vector.tensor_copy(
                            v_sb[0:mt, t, :, :].rearrange(
                                "p (par a) d -> p a par d", par=2)[:, 0:4, :, :],
                            vp[0:mt, :].rearrange("p (a par d) -> p a par d",
                                                  par=2, d=HEAD_DIM),
                        )
                        nc.scalar.copy(
                            v_sb[0:mt, t, :, :].rearrange(
                                "p (par a) d -> p a par d", par=2)[:, 4:6, :, :],
                            vp2[0:mt, :].rearrange("p (a par d) -> p a par d",
                                                   par=2, d=HEAD_DIM),
                        )

                sums_tiles = {}
                po_tiles = {0: {}, 1: {}}

                def attn_phase(hb):
                    # ---- attention, with prev-batch proj interleaved ----
                    b = 2 * g + hb
                    v_sb = v_tiles[hb]
                    sums_pA = ps_sums.tile([65, 512], F32, tag="sums", name=f"sumsA_{b}")
                    sums_pB = ps_sums.tile([65, 512], F32, tag="sums", name=f"sumsB_{b}")
                    sums_tiles[hb] = (sums_pA, sums_pB)
                    po_sb_by_pair = po_tiles[hb]
                    for step, sp in enumerate([0, 3, 1, 4, 2, 5]):
                        h0, h1 = PAIRS[sp]
                        expT = sb_exp.tile([128, 2, 2 * NPAD], F32R, tag="expT",
                                           name=f"expT_{b}_{sp}")
                        sums_px = sums_pA if sp < 3 else sums_pB
                        j3 = sp % 3
                        scs = []
                        for t, (off, mt) in enumerate(TOK_TILES):
                            sc = ps.tile([128, 512], F32, tag="ps", name=f"sc_{b}_{sp}_{t}")
                            nc.tensor.matmul(
                                sc[0:mt, 0:NPAD], kT(qkT_sb, h0, hb, off, mt),
                                qT(qkT_sb, h0, hb), start=True, stop=True,
                            )
                            nc.tensor.matmul(
                                sc[0:mt, NPAD:512], kT(qkT_sb, h1, hb, off, mt),
                                qT(qkT_sb, h1, hb), start=True, stop=True,
                            )
                            scs.append(sc)
                        for t, (off, mt) in enumerate(TOK_TILES):
                            nc.scalar.activation(
                                expT[0:mt, t, :], scs[t][0:mt, :],
                                mybir.ActivationFunctionType.Exp,
                            )
                            nc.vector.tensor_mul(
                                expT[0:mt, t, :], expT[0:mt, t, :],
                                relt_sb[0:mt, sp, t, :],
                            )
                        # long warm proj matmuls of the previous batch fill the
                        # exp/mult wait and keep the PE clock-gate open
                        emit_proj_chunk(step)
                        po = ps.tile([128, 512], F32, tag="ps", name=f"po_{b}_{sp}")
                        for t, (off, mt) in enumerate(TOK_TILES):
                            nc.tensor.matmul(
                                po[:, :], v_sb[0:mt, t, 2 * sp:2 * sp + 2, :],
                                expT[0:mt, t, :], start=(t == 0), stop=(t == 1),
                            )
                            nc.tensor.matmul(
                                sums_px[0:65, :], oh3_sb[0:mt, j3, :], expT[0:mt, t, :],
                                start=(sp in (0, 3) and t == 0),
                                stop=(sp in (2, 5) and t == 1),
                                skip_group_check=True,
                            )
                        po_sb = sb_po.tile([128, NPAD], F32, tag="po",
                                           name=f"po_sb_{b}_{sp}")
                        nc.scalar.copy(po_sb[0:64, :], po[0:64, 0:NPAD])
                        nc.vector.tensor_copy(po_sb[64:128, :], po[64:128, NPAD:512])
                        po_sb_by_pair[sp] = po_sb

                def norm_phase(hb):
                    b = 2 * g + hb
                    sums_pA, sums_pB = sums_tiles[hb]
                    po_sb_by_pair = po_tiles[hb]
                    rec_fA = sb_rec.tile([65, 512], F32, tag="recf", name=f"recfA_{b}")
                    rec_fB = sb_rec.tile([65, 512], F32, tag="recf", name=f"recfB_{b}")
                    nc.vector.reciprocal_approx_fast(out=rec_fA[0:65, :],
                                                     in_=sums_pA[0:65, :])
                    nc.vector.reciprocal_approx_fast(out=rec_fB[0:65, :],
                                                     in_=sums_pB[0:65, :])
                    rec_sbA = sb_rec.tile([65, 512], F32R, tag="rec", name=f"recA_{b}")
                    rec_sbB = sb_rec.tile([65, 512], F32R, tag="rec", name=f"recB_{b}")
                    nc.scalar.copy(rec_sbA[0:65, :], rec_fA[0:65, :])
                    nc.scalar.copy(rec_sbB[0:65, :], rec_fB[0:65, :])

                    cT_sb = sb_cT.tile([128, KT, NPAD], F32R, tag="cT", name=f"cT_{b}")
                    for sp in range(6):
                        rec_x = rec_sbA if sp < 3 else rec_sbB
                        r0 = 32 * (sp % 3)
                        pb2 = ps.tile([64, 512], F32, tag="ps", name=f"pb_{b}_{sp}")
                        nc.tensor.matmul(
                            pb2[0:64, :], allones_sb[r0:r0 + 1, 0:64],
                            rec_x[r0:r0 + 1, 0:512],
                            start=True, stop=True,
                        )
                        pblo = sb_ctmp.tile([64, NPAD], F32, tag="ctmp",
                                            name=f"pblo_{b}_{sp}")
                        nc.scalar.copy(pblo[:], pb2[0:64, NPAD:512])
                        pbhi = sb_pbs.tile([128, NPAD], F32, tag="pbs",
                                           name=f"pbhi_{b}_{sp}")
                        nc.sync.dma_start(out=pbhi[64:128, :], in_=pblo[:])
                        po_sb = po_sb_by_pair[sp]
                        nc.vector.tensor_mul(cT_sb[0:64, sp, :], po_sb[0:64, :],
                                             pb2[0:64, 0:NPAD])
                        nc.vector.tensor_mul(cT_sb[64:128, sp, :], po_sb[64:128, :],
                                             pbhi[64:128, :])
                    prev_proj[0] = (b, cT_sb)

                # v(1) is emitted before norm(0) so its matmuls fill the PE
                # bubble while the DVE runs batch 0's reciprocal chain
                v_phase(0)
                attn_phase(0)
                v_phase(1)
                norm_phase(0)
                attn_phase(1)
                norm_phase(1)

            # drain the last batch's projection
            emit_proj_chunk(0, drain=True)

    nc.compile()
    nc.m = get_hw_module(nc.m)
    return nc


def _host_prep(x, qkv_weight, q_bias, v_bias, rel_table, proj_weight, proj_bias,
               b_idx, rel_index):
    x = np.asarray(x, dtype=np.float32)
    # xt8[b, k, p, n] = x[b, n, 128k+p], zero-padded to NPAD tokens
    xt = np.zeros((B, KT, 128, NPAD), dtype=np.float32)
    xt[:, :, :, 0:N] = x.transpose(0, 2, 1).reshape(B, KT, 128, N)
    W = np.asarray(qkv_weight, dtype=np.float32).copy()
    W[:DIM] *= np.float32(SCALE)
    wt = np.ascontiguousarray(W.T.reshape(KT, 128, 3 * DIM))
    pwtT = np.asarray(proj_weight, dtype=np.float32).T  # [c', o]
    pwtT = pwtT.reshape(HEADS, HEAD_DIM, DIM)[PAIR_PERM].reshape(DIM, DIM)
    pwt = np.ascontiguousarray(pwtT.reshape(KT, 128, DIM))

    bi = np.asarray(b_idx).astype(np.int64)
    qb_all = (np.asarray(q_bias, dtype=np.float32)[bi] * np.float32(SCALE))
    vb_all = np.asarray(v_bias, dtype=np.float32)[bi]
    # softmax rows sum to 1, so attn @ (1 x vb) == 1 x vb; push the v bias
    # through the projection into the proj bias
    pb_all = (np.asarray(proj_bias, dtype=np.float32)[bi]
              + vb_all @ np.asarray(proj_weight, dtype=np.float32).T)

    ridx = np.asarray(rel_index).astype(np.int64)
    rel = np.asarray(rel_table, dtype=np.float32)[ridx.reshape(-1)]
    rel = rel.reshape(N, N, HEADS)  # [n, m, h]
    relth = np.zeros((HEADS, 2, 128, NPAD), dtype=np.float32)
    for t, (off, mt) in enumerate(TOK_TILES):
        # relth[h, t, p, n] = exp(rel[n, off+p, h])
        relth[:, t, 0:mt, 0:N] = np.exp(rel[:, off:off + mt, :].transpose(2, 1, 0))
    # pair-merged: relt[sp, t, p, i*NPAD+n] = relth[PAIRS[sp][i], t, p, n]
    relt = np.ascontiguousarray(
        relth[PAIR_PERM].reshape(6, 2, 2, 128, NPAD)
        .transpose(0, 2, 3, 1, 4).reshape(6, 2, 128, 2 * NPAD))

    oh3 = np.zeros((128, 3, 65), dtype=np.float32)
    for j in range(3):
        oh3[:, j, 32 * j] = 1.0
    allones = np.ones((128, 64), dtype=np.float32)

    in_maps = []
    for c in range(NCORES):
        sl = slice(c * BPC, (c + 1) * BPC)
        qbc = np.ascontiguousarray(
            qb_all[sl].reshape(BPC, KT, 128).transpose(2, 0, 1))
        in_maps.append({
            "xt8": np.ascontiguousarray(xt[sl]),
            "wt": wt,
            "pwt": pwt,
            "qbc": qbc,
            "relt": relt,
            "oh3": oh3,
            "allones": allones,
        })
    return in_maps, pb_all


def _install_ntff_hook():
    """Provide antenv.axon_hooks (absent from this image) so bass_utils can
    capture NTFF profiles through libaxon_pjrt.so, and keep artifacts local."""
    if _CACHE.get("hook_installed"):
        return
    import sys
    import types
    import ctypes
    import contextlib

    so_path = "/opt/axon/libaxon_pjrt.so"
    lib = ctypes.CDLL(so_path)
    lib.axon_start_nrt_profile.argtypes = [
        ctypes.POINTER(ctypes.c_int64),
        ctypes.c_size_t,
    ]
    lib.axon_start_nrt_profile.restype = ctypes.c_int64
    lib.axon_stop_nrt_profile.argtypes = [ctypes.c_char_p]
    lib.axon_stop_nrt_profile.restype = ctypes.c_int64

    @contextlib.contextmanager
    def _hook(output_dir, device_ids):
        import jax

        jax.devices()
        if device_ids:
            ids = (ctypes.c_int64 * len(device_ids))(*device_ids)
            rc = lib.axon_start_nrt_profile(ids, len(device_ids))
        else:
            rc = lib.axon_start_nrt_profile(None, 0)
        if rc != 0:
            raise RuntimeError(f"axon_start_nrt_profile rc={rc}")
        try:
            yield
        finally:
            n = lib.axon_stop_nrt_profile(str(output_dir).encode())
            print(f"ntff profile: {n} file(s) written to {output_dir}")

    mod = types.ModuleType("antenv.axon_hooks")
    mod.get_axon_ntff_profile_hook = lambda: _hook
    mod.set_axon_ntff_profile_hook = lambda h: None
    sys.modules["antenv.axon_hooks"] = mod

    import concourse.bass_utils as bu

    bu.upload_artifacts = lambda tmpdir: str(tmpdir)
    _CACHE["hook_installed"] = True


def kernel(**inputs):
    if "nc" not in _CACHE:
        _CACHE["nc"] = _build_module()
    nc = _CACHE["nc"]

    in_maps, pb_all = _host_prep(**inputs)
    trace = os.environ.get("KERNEL_TRACE", "0") == "1"
    tmpdir = None
    if trace:
        _install_ntff_hook()
        tmpdir = os.environ.get("KERNEL_TRACE_DIR") or None
    res = run_bass_kernel_spmd(nc, in_maps, core_ids=list(range(NCORES)), trace=trace,
                               tmpdir=tmpdir)
    if trace:
        _CACHE["last_exec_time_ns"] = res.exec_time_ns
        _CACHE["last_results"] = res

    y = np.concatenate([res.results[c]["y8"] for c in range(NCORES)], axis=0)
    y += pb_all[:, None, :]
    return y



# revision 16
# speedup vs baseline: 1.2868x; 1.2297x over previous
"""Trainium2 Bass kernel for BEiT attention block (nn_Beit_9560597201107).

Data-parallel over batch: 64 batches -> 8 NeuronCores x 8 batches each.
Transposed dataflow (channels on partitions) so the softmax'd attention
matrix is never transposed on-chip; batch PAIRS are packed to 394 token
columns (>=256 keeps fp32r at 1 cycle/row) for the qkv and output
projections, and the post-exp path runs in bf16 (full rate at any free
size, so token dims shrink to the exact 197):

  qkT[c, 2x197] = sum_k WT[k, c] xT[k, :] + qbias     fp32r, pair-packed
  v[m, (sp, i, d|1)]  = sum_k xT[k, m] WT_v[k, d]     65th column = ones
  scT_h[m, n] = sum_d kT[d, m] qT[d, n]               fp32r; the two heads
      of a pair sit at partition halves 0:64 / 64:128 -> concurrent
      row-tiled matmuls into separate PSUM banks
  eT = exp(scT) * exp_rel_T                           bf16 [m, 2x197]
  poA[0:65, 2x197] = [v_h0 | 1]^T eT                  row 64 = softmax sums
  poB[64:128, 197] = v_h1^T eT_h1                     col-tiled to (0,64)
  pb = ones-outer-product broadcast of 1/sums         bf16 matmuls, N=197
  cT = po * pb;   yT[o, 2x197] = sum_c pwT[c, o] cT[c, :] + bias(partition)

Projection of pair g is emitted during pair g+1's qkT/v phases; scores are
software-pipelined one head-pair ahead so the PE never waits on the exp.
"""

import os
import numpy as np
from ml_dtypes import bfloat16

import concourse.bass as bass
import concourse.bacc as bacc
import concourse.mybir as mybir
import concourse.tile as tile
from concourse.bass_utils import run_bass_kernel_spmd
from concourse.bass_interp import get_hw_module

B, N, DIM, HEADS, NBS = 64, 197, 768, 12, 10
HEAD_DIM = DIM // HEADS
SCALE = HEAD_DIM ** -0.5
NCORES = 8
BPC = B // NCORES          # batches per core
NPAIR = BPC // 2
KT = DIM // 128            # 6 contraction tiles
N2 = 2 * N                 # 394: batch-pair-packed token columns
TOK_TILES = [(0, 128), (128, 69)]  # (offset, size) over the 197 tokens

F32 = mybir.dt.float32
F32R = mybir.dt.float32r
BF16 = mybir.dt.bfloat16
IDENT = mybir.ActivationFunctionType.Identity
EXP = mybir.ActivationFunctionType.Exp

_CACHE = {}


def _build_module():
    nc = bacc.Bacc("TRN2", target_bir_lowering=False, debug=False)

    xt_d = nc.dram_tensor("xt", [NPAIR, 128, KT, N2], F32, kind="ExternalInput")
    wtq_d = nc.dram_tensor("wtq", [6, 128, KT, 128], F32, kind="ExternalInput")
    wtk_d = nc.dram_tensor("wtk", [6, 128, KT, 128], F32, kind="ExternalInput")
    wtv_d = nc.dram_tensor("wtv", [128, KT, DIM], F32, kind="ExternalInput")
    pwt_d = nc.dram_tensor("pwt", [128, KT, DIM], BF16, kind="ExternalInput")
    relt_d = nc.dram_tensor("relt", [128, 6, 2, N2], BF16, kind="ExternalInput")
    qbc_d = nc.dram_tensor("qbc", [128, BPC, KT], F32, kind="ExternalInput")
    vpbt_d = nc.dram_tensor("vpbt", [128, KT, BPC], F32, kind="ExternalInput")
    aon_d = nc.dram_tensor("aon", [65, 64], BF16, kind="ExternalInput")
    von_d = nc.dram_tensor("von", [128, 12], BF16, kind="ExternalInput")
    yt_d = nc.dram_tensor("yt", [NPAIR, KT, 128, N2], F32, kind="ExternalOutput")
    dbg = os.environ.get("KERNEL_DEBUG", "0") == "1"
    if dbg:
        dqkT_d = nc.dram_tensor("dqkT", [128, 12, N2], F32, kind="ExternalOutput")
        dexp_d = nc.dram_tensor("dexp", [12, 128, 2, N2], F32, kind="ExternalOutput")
        dv_d = nc.dram_tensor("dv", [2, 128, 2, KT, 2, 65], F32, kind="ExternalOutput")
        dcT_d = nc.dram_tensor("dcT", [128, KT, N2], F32, kind="ExternalOutput")
        dpoA_d = nc.dram_tensor("dpoA", [128, 512], F32, kind="ExternalOutput")
        dpoB_d = nc.dram_tensor("dpoB", [128, 512], F32, kind="ExternalOutput")
        drec_d = nc.dram_tensor("drec", [1, N2], F32, kind="ExternalOutput")
        drecf_d = nc.dram_tensor("drecf", [1, N2], F32, kind="ExternalOutput")
        dpb_d = nc.dram_tensor("dpb", [128, N], F32, kind="ExternalOutput")

    with tile.TileContext(nc) as tc:
        with (
            tc.tile_pool(name="const", bufs=1) as constp,
            tc.tile_pool(name="sb_xT", bufs=2) as sb_xT,
            tc.tile_pool(name="sb_qkT", bufs=2) as sb_qkT,
            tc.tile_pool(name="sb_v", bufs=2) as sb_v,
            tc.tile_pool(name="sb_exp", bufs=3) as sb_exp,
            tc.tile_pool(name="sb_rec", bufs=6) as sb_rec,
            tc.tile_pool(name="sb_pb", bufs=4) as sb_pb,
            tc.tile_pool(name="sb_cT", bufs=2) as sb_cT,
            tc.tile_pool(name="sb_out", bufs=3) as sb_out,
            tc.tile_pool(name="ps", bufs=8, space="PSUM") as ps,
        ):
            # ---- persistent data, streamed in consumption order ----
            # gpsimd queue (fp32 -> fp32r casting loads): wtq ct0, x pair0,
            # wtq rest, wtk, wtv, x pair1. sync queue (no cast): the small
            # bf16/f32 tables, racing ahead in parallel.
            wtq_sb = constp.tile([128, 6, KT, 128], F32R)
            wtk_sb = constp.tile([128, 6, KT, 128], F32R)
            nc.gpsimd.dma_start(out=wtq_sb[:, 0], in_=wtq_d.ap()[0])

            xT_tiles = {}

            def load_xT(g):
                t_ = sb_xT.tile([128, KT, N2], F32R, tag="xT", name=f"xT_{g}")
                nc.gpsimd.dma_start(out=t_[:], in_=xt_d.ap()[g])
                xT_tiles[g] = t_

            load_xT(0)
            for ct in range(1, 6):
                nc.gpsimd.dma_start(out=wtq_sb[:, ct], in_=wtq_d.ap()[ct])
            for ct in range(6):
                nc.gpsimd.dma_start(out=wtk_sb[:, ct], in_=wtk_d.ap()[ct])
            wtv_sb = constp.tile([128, KT, DIM], F32R)
            nc.gpsimd.dma_start(out=wtv_sb[:], in_=wtv_d.ap())
            load_xT(1)

            qbc_sb = constp.tile([128, BPC, KT], F32)
            nc.sync.dma_start(out=qbc_sb[:], in_=qbc_d.ap())
            vpbt_sb = constp.tile([128, KT, BPC], F32)
            nc.sync.dma_start(out=vpbt_sb[:], in_=vpbt_d.ap())
            aon_sb = constp.tile([65, 64], BF16)
            nc.sync.dma_start(out=aon_sb[:], in_=aon_d.ap())
            von_sb = constp.tile([128, 12], BF16)
            nc.sync.dma_start(out=von_sb[:], in_=von_d.ap())
            relt_sb = constp.tile([128, 6, 2, N2], BF16)
            nc.sync.dma_start(out=relt_sb[:], in_=relt_d.ap())
            pwt_sb = constp.tile([128, KT, DIM], BF16)
            nc.sync.dma_start(out=pwt_sb[:], in_=pwt_d.ap())

            # ---- transposed pair-packed output projection ----
            def emit_projT(src, o, c0, c1):
                gs, cT_ = src
                w = c1 - c0
                prT = ps.tile([128, 512], F32, tag="ps", name=f"prT_{gs}_{o}_{c0}")
                for c in range(KT):
                    nc.tensor.matmul(
                        prT[:, 0:w], pwt_sb[:, c, o * 128:(o + 1) * 128],
                        cT_[:, c, c0:c1], start=(c == 0), stop=(c == KT - 1),
                    )
                out_sb = sb_out.tile([128, N2], F32, tag="out",
                                     name=f"out_{gs}_{o}_{c0}")
                if c0 == 0:
                    nc.scalar.activation(out_sb[:, 0:N], prT[:, 0:N], IDENT,
                                         bias=vpbt_sb[:, o, 2 * gs:2 * gs + 1])
                if c1 == N2:
                    nc.vector.tensor_scalar_add(
                        out_sb[:, N:N2], prT[:, w - N:w],
                        vpbt_sb[:, o, 2 * gs + 1:2 * gs + 2])
                nc.sync.dma_start(out=yt_d.ap()[gs, o, :, c0:c1],
                                  in_=out_sb[:, c0:c1])

            prev = [None]

            for g in range(NPAIR):
                xT_sb = xT_tiles.pop(g)

                # ---- qkT for the pair: 12 col-chunks x 6 k, N=394 fp32r ----
                qkT_sb = sb_qkT.tile([128, 12, N2], F32R, tag="qkT",
                                     name=f"qkT_{g}")
                for ct in range(12):
                    w = wtq_sb if ct < 6 else wtk_sb
                    qp = ps.tile([128, 512], F32, tag="ps", name=f"qp_{g}_{ct}")
                    for k in range(KT):
                        nc.tensor.matmul(
                            qp[:, 0:N2], w[:, ct % 6, k, :], xT_sb[:, k, :],
                            start=(k == 0), stop=(k == KT - 1),
                        )
                    for hb in range(2):
                        dst = qkT_sb[:, ct, hb * N:(hb + 1) * N]
                        src = qp[:, hb * N:(hb + 1) * N]
                        if ct < 6:
                            qb = qbc_sb[:, 2 * g + hb, ct:ct + 1]
                            if hb == 0:
                                nc.vector.tensor_scalar_add(dst, src, qb)
                            else:
                                nc.scalar.activation(dst, src, IDENT, bias=qb)
                        else:
                            if hb == 0:
                                nc.vector.tensor_copy(dst, src)
                            else:
                                nc.scalar.copy(dst, src)

                if dbg and g == 0:
                    nc.gpsimd.dma_start(out=dqkT_d.ap(), in_=qkT_sb[:, :, :])

                if g + 2 < NPAIR:
                    load_xT(g + 2)

                # previous pair's projection rides the dense qkT/v stretch
                if prev[0] is not None:
                    for o in range(3):
                        emit_projT(prev[0], o, 0, N2)

                cT_pair = sb_cT.tile([128, KT, N2], BF16, tag="cT", name=f"cT_{g}")
                v_tiles = {}

                def v_phase(hb):
                    b = 2 * g + hb
                    v_sb = sb_v.tile([128, 2, KT, 2, 65], BF16, tag="v",
                                     name=f"v_{b}")
                    v_tiles[hb] = v_sb
                    for t, (off, mt) in enumerate(TOK_TILES):
                        nc.gpsimd.tensor_copy(
                            v_sb[:, t, :, :, 64:65],
                            von_sb[:, 0:12].rearrange("p (a i o) -> p a i o",
                                                      i=2, o=1),
                        )
                        vp = ps.tile([128, 512], F32, tag="ps", name=f"vp_{b}_{t}")
                        vp2 = ps.tile([128, 512], F32, tag="ps", name=f"vp2_{b}_{t}")
                        for k in range(KT):
                            xsl = xT_sb[:, k, hb * N + off:hb * N + off + mt]
                            nc.tensor.matmul(
                                vp[0:mt, 0:512], xsl, wtv_sb[:, k, 0:512],
                                start=(k == 0), stop=(k == KT - 1),
                            )
                            nc.tensor.matmul(
                                vp2[0:mt, 0:256], xsl, wtv_sb[:, k, 512:768],
                                start=(k == 0), stop=(k == KT - 1),
                            )
                        nc.vector.tensor_copy(
                            v_sb[0:mt, t, 0:4, 0:2, 0:64],
                            vp[0:mt, 0:512].rearrange(
                                "p (a i d) -> p a i d", i=2, d=64),
                        )
                        nc.scalar.copy(
                            v_sb[0:mt, t, 4:6, 0:2, 0:64],
                            vp2[0:mt, 0:256].rearrange(
                                "p (a i d) -> p a i d", i=2, d=64),
                        )
                    if dbg and g == 0:
                        nc.gpsimd.dma_start(out=dv_d.ap()[hb], in_=v_sb[:])

                def attn_phase(hb, interleave=None):
                    b = 2 * g + hb
                    v_sb = v_tiles[hb]
                    sc_tiles = {}

                    q0 = hb * 138   # query window start: [0,256) or [138,394)
                    sk = hb * 59    # in-window offset of this batch's queries

                    def emit_sc(sp):
                        sc0 = ps.tile([128, 512], F32, tag="ps",
                                      name=f"sc0_{b}_{sp}")
                        sc1 = ps.tile([128, 512], F32, tag="ps",
                                      name=f"sc1_{b}_{sp}")
                        for t, (off, mt) in enumerate(TOK_TILES):
                            nc.tensor.matmul(
                                sc0[0:mt, t * 256:t * 256 + 256],
                                qkT_sb[0:64, 6 + sp, hb * N + off:hb * N + off + mt],
                                qkT_sb[0:64, sp, q0:q0 + 256],
                                start=True, stop=True,
                            )
                            nc.tensor.matmul(
                                sc1[0:mt, t * 256:t * 256 + 256],
                                qkT_sb[64:128, 6 + sp, hb * N + off:hb * N + off + mt],
                                qkT_sb[64:128, sp, q0:q0 + 256],
                                start=True, stop=True,
                            )
                        sc_tiles[sp] = (sc0, sc1)

                    emit_sc(0)
                    for sp in range(6):
                        # scores one pair ahead: PE computes sp+1's scores
                        # while the scalar engine runs sp's exp
                        if sp + 1 < 6:
                            emit_sc(sp + 1)
                        sc0, sc1 = sc_tiles.pop(sp)
                        expT = sb_exp.tile([128, 2, N2], BF16, tag="expT",
                                           name=f"expT_{b}_{sp}")
                        for t, (off, mt) in enumerate(TOK_TILES):
                            nc.scalar.activation(
                                expT[0:mt, t, 0:N],
                                sc0[0:mt, t * 256 + sk:t * 256 + sk + N], EXP)
                            nc.scalar.activation(
                                expT[0:mt, t, N:N2],
                                sc1[0:mt, t * 256 + sk:t * 256 + sk + N], EXP)
                            nc.vector.tensor_mul(
                                expT[0:mt, t, :], expT[0:mt, t, :],
                                relt_sb[0:mt, sp, t, :],
                            )
                        if dbg and g == 0:
                            nc.gpsimd.dma_start(out=dexp_d.ap()[hb * 6 + sp],
                                                in_=expT[:])
                        if interleave is not None:
                            interleave(sp)
                        poA = ps.tile([128, 512], F32, tag="ps", name=f"poA_{b}_{sp}")
                        poB = ps.tile([128, 512], F32, tag="ps", name=f"poB_{b}_{sp}")
                        for t, (off, mt) in enumerate(TOK_TILES):
                            nc.tensor.matmul(
                                poA[0:65, 0:N2], v_sb[0:mt, t, sp, 0, 0:65],
                                expT[0:mt, t, :], start=(t == 0), stop=(t == 1),
                            )
                            nc.tensor.matmul(
                                poB[64:128, 0:N], v_sb[0:mt, t, sp, 1, 0:64],
                                expT[0:mt, t, N:N2],
                                start=(t == 0), stop=(t == 1),
                            )
                        sums_b = sb_rec.tile([65, N2], BF16, tag="sums",
                                             name=f"sums_{b}_{sp}")
                        nc.scalar.copy(sums_b[64:65, :], poA[64:65, 0:N2])
                        pb = ps.tile([128, 512], F32, tag="ps", name=f"pb_{b}_{sp}")
                        nc.tensor.matmul(pb[0:64, 0:N], aon_sb[64:65, 0:64],
                                         sums_b[64:65, 0:N], start=True, stop=True)
                        nc.tensor.matmul(pb[64:128, 0:N], aon_sb[64:65, 0:64],
                                         sums_b[64:65, N:N2], start=True, stop=True)
                        pb_sb = sb_pb.tile([128, N], F32, tag="pb",
                                           name=f"pb_{b}_{sp}")
                        nc.vector.reciprocal_approx_fast(
                            out=pb_sb[0:128, :], in_=pb[0:128, 0:N])
                        if dbg and b == 0 and sp == 0:
                            tA = sb_out.tile([128, 512], F32, name="dbg_tA")
                            nc.vector.tensor_copy(tA[:], poA[:, :])
                            nc.gpsimd.dma_start(out=dpoA_d.ap(), in_=tA[:])
                            tB = sb_out.tile([128, 512], F32, name="dbg_tB")
                            nc.vector.tensor_copy(tB[:], poB[:, :])
                            nc.gpsimd.dma_start(out=dpoB_d.ap(), in_=tB[:])
                            nc.gpsimd.dma_start(out=dpb_d.ap(), in_=pb_sb[:])
                        nc.vector.tensor_mul(
                            cT_pair[0:64, sp, hb * N:(hb + 1) * N],
                            poA[0:64, 0:N], pb_sb[0:64, :])
                        nc.vector.tensor_mul(
                            cT_pair[64:128, sp, hb * N:(hb + 1) * N],
                            poB[64:128, 0:N], pb_sb[64:128, :])

                v_phase(0)
                if prev[0] is not None:
                    for o in range(3, 6):
                        emit_projT(prev[0], o, 0, N2)
                attn_phase(0)
                v_phase(1)
                if g == NPAIR - 1:
                    # tail: batch A's projection interleaves into batch B's
                    # attention; only batch B's half drains at the end
                    attn_phase(1, interleave=lambda sp: emit_projT(
                        (g, cT_pair), sp, 0, N))
                else:
                    attn_phase(1)
                if dbg and g == 0:
                    nc.gpsimd.dma_start(out=dcT_d.ap(), in_=cT_pair[:])
                prev[0] = (g, cT_pair)

            for o in range(KT):
                emit_projT(prev[0], o, N, N2)

    nc.compile()
    nc.m = get_hw_module(nc.m)
    return nc


def _host_prep(x, qkv_weight, q_bias, v_bias, rel_table, proj_weight, proj_bias,
               b_idx, rel_index):
    x = np.asarray(x, dtype=np.float32)
    W = np.asarray(qkv_weight, dtype=np.float32).copy()
    W[:DIM] *= np.float32(SCALE)
    WT = np.ascontiguousarray(W.T)               # [cin, cout]
    wtq = np.ascontiguousarray(
        WT[:, 0:DIM].reshape(KT, 128, 6, 128).transpose(2, 1, 0, 3))
    wtk = np.ascontiguousarray(
        WT[:, DIM:2 * DIM].reshape(KT, 128, 6, 128).transpose(2, 1, 0, 3))
    wtv = np.ascontiguousarray(
        WT[:, 2 * DIM:].reshape(KT, 128, DIM).transpose(1, 0, 2))
    pwtT = np.asarray(proj_weight, dtype=np.float32).T   # [cin, cout]
    pwt = np.ascontiguousarray(
        pwtT.reshape(KT, 128, DIM).transpose(1, 0, 2)).astype(bfloat16)

    bi = np.asarray(b_idx).astype(np.int64)
    qb_all = np.asarray(q_bias, dtype=np.float32)[bi] * np.float32(SCALE)
    vb_all = np.asarray(v_bias, dtype=np.float32)[bi]
    # softmax rows sum to 1, so attn @ (1 x vb) == 1 x vb; push the v bias
    # through the projection into the proj bias
    pb_all = (np.asarray(proj_bias, dtype=np.float32)[bi]
              + vb_all @ np.asarray(proj_weight, dtype=np.float32).T)

    ridx = np.asarray(rel_index).astype(np.int64)
    relE = np.exp(np.asarray(rel_table, dtype=np.float32)[ridx.reshape(-1)]
                  .reshape(N, N, HEADS))           # [n, m, h]
    relM = relE.transpose(1, 0, 2)                  # [m, n, h]
    relt = np.zeros((128, 6, 2, N2), dtype=np.float32)
    for t, (off, mt) in enumerate(TOK_TILES):
        seg = relM[off:off + mt]                    # [mt, n, h]
        relt[0:mt, :, t, :] = (seg.reshape(mt, N, 6, 2)
                               .transpose(0, 2, 3, 1).reshape(mt, 6, N2))
    relt = relt.astype(bfloat16)
    aon = np.ones((65, 64), dtype=bfloat16)
    von = np.ones((128, 12), dtype=bfloat16)

    in_maps = []
    for c in range(NCORES):
        sl = slice(c * BPC, (c + 1) * BPC)
        xs = x[sl]                                  # [8, 197, 768]
        xt = np.ascontiguousarray(
            xs.reshape(NPAIR, 2, N, DIM).transpose(0, 3, 1, 2)
            .reshape(NPAIR, KT, 128, N2).transpose(0, 2, 1, 3))
        qbc = np.ascontiguousarray(
            qb_all[sl].reshape(BPC, KT, 128).transpose(2, 0, 1))
        vpbt = np.ascontiguousarray(
            pb_all[sl].reshape(BPC, KT, 128).transpose(2, 1, 0))
        in_maps.append({
            "xt": xt,
            "wtq": wtq,
            "wtk": wtk,
            "wtv": wtv,
            "pwt": pwt,
            "relt": relt,
            "qbc": qbc,
            "vpbt": vpbt,
            "aon": aon,
            "von": von,
        })
    return in_maps


def _install_ntff_hook():
    """Provide antenv.axon_hooks (absent from this image) so bass_utils can
    capture NTFF profiles through libaxon_pjrt.so, and keep artifacts local."""
    if _CACHE.get("hook_installed"):
        return
    import sys
    import types
    import ctypes
    import contextlib

    so_path = "/opt/axon/libaxon_pjrt.so"
    lib = ctypes.CDLL(so_path)
    lib.axon_start_nrt_profile.argtypes = [
        ctypes.POINTER(ctypes.c_int64),
        ctypes.c_size_t,
    ]
    lib.axon_start_nrt_profile.restype = ctypes.c_int64
    lib.axon_stop_nrt_profile.argtypes = [ctypes.c_char_p]
    lib.axon_stop_nrt_profile.restype = ctypes.c_int64

    @contextlib.contextmanager
    def _hook(output_dir, device_ids):
        import jax

        jax.devices()
        if device_ids:
            ids = (ctypes.c_int64 * len(device_ids))(*device_ids)
            rc = lib.axon_start_nrt_profile(ids, len(device_ids))
        else:
            rc = lib.axon_start_nrt_profile(None, 0)
        if rc != 0:
            raise RuntimeError(f"axon_start_nrt_profile rc={rc}")
        try:
            yield
        finally:
            n = lib.axon_stop_nrt_profile(str(output_dir).encode())
            print(f"ntff profile: {n} file(s) written to {output_dir}")

    mod = types.ModuleType("antenv.axon_hooks")
    mod.get_axon_ntff_profile_hook = lambda: _hook
    mod.set_axon_ntff_profile_hook = lambda h: None
    sys.modules["antenv.axon_hooks"] = mod

    import concourse.bass_utils as bu

    bu.upload_artifacts = lambda tmpdir: str(tmpdir)
    _CACHE["hook_installed"] = True


def kernel(**inputs):
    if "nc" not in _CACHE:
        _CACHE["nc"] = _build_module()
    nc = _CACHE["nc"]

    in_maps = _host_prep(**inputs)
    trace = os.environ.get("KERNEL_TRACE", "0") == "1"
    tmpdir = None
    if trace:
        _install_ntff_hook()
        tmpdir = os.environ.get("KERNEL_TRACE_DIR") or None
    res = run_bass_kernel_spmd(nc, in_maps, core_ids=list(range(NCORES)), trace=trace,
                               tmpdir=tmpdir)
    if trace:
        _CACHE["last_exec_time_ns"] = res.exec_time_ns
        _CACHE["last_results"] = res

    ys = []
    for c in range(NCORES):
        yt = np.asarray(res.results[c]["yt"])       # [4, 6, 128, 394]
        ys.append(yt.reshape(NPAIR, KT, 128, 2, N)
                  .transpose(0, 3, 4, 1, 2).reshape(BPC, N, DIM))
    return np.ascontiguousarray(np.concatenate(ys, axis=0), dtype=np.float32)
